# revision 11
# baseline (speedup 1.0000x reference)
"""Trainium2 Bass kernel for nn_MCMCSampler.

Math: the energy gradient w.r.t. preds is purely elementwise (the feature
einsum is constant w.r.t. preds so it drops out of jax.grad):

    p     = sigmoid(x)
    grad  = c * p(1-p) * (w + beta*L),   c[b,h] = mask[b,h]/(horses[b]*V*B)
    x    <- x - STEP*grad*mask

where L = dentropy/dp collapses to x (logit o sigmoid identity, eps terms
cancel at O(eps^2)). The update is ~8e-10 per step against x ~ 0.1, so
delta is constant across the 16 steps to ~1e-16: compute delta once from
x0 and chain x_t = x_{t-1} - delta.

Precision: fp16 end to end. sigmoid'(x) = p(1-p) is evaluated via its
Taylor series 1/4 - x^2/16 (|x| <= ~0.6 here; the series error lands on
a delta that only moves the output at the 1e-9 level, far below fp16
resolution). fp16 halves DMA traffic and unlocks the DVE 2x_1p/4x_2p
perf modes. Output error vs the fp32 reference is the fp16 quantization
floor, ~2e-4 rel.

Sharding: data-parallel over V (64 variants / 8 cores), no cross-core
communication. Per-core output: [16, 128*1536] fp16 = 6.3 MB.

Schedule (CoreSim v1 cost model): a DMA occupies its issuing engine
queue (SP / ACT HWDGE, Pool SWDGE) for bytes/332GB/s; queues run
concurrently; a DMA's data is visible to the *issuing* engine's later
instructions at transfer end but to other engines only ~1.7us later
(sem path). Exploits:
  - exactly ONE column-chunk per engine, so the chain's data deps pin
    the per-engine instruction order (no compile-time round-robin
    interleave -> no head-of-line blocking);
  - Pool loads its own x0 share through its SWDGE queue and starts
    chaining ~0.6us in, vs 2.4us for the DVE share (SP load + sem);
  - per-step DMA slabs assigned to SP/ACT (and Pool once its chains
    end) by a small beam search over the measured cost constants.
"""

import numpy as np
from contextlib import ExitStack

import concourse.bass as bass
from concourse import bacc
import concourse.mybir as mybir
import concourse.tile as tile
from concourse.bass_utils import run_bass_kernel_spmd

NCORES = 8
V, B, H = 64, 1024, 24
S = 16
STEP_SIZE = 0.1
BETA = 0.1
VSH = V // NCORES          # 8 variants per core
N = VSH * B * H            # 196608 elements per core
P = 128                    # SBUF partitions
F = N // P                 # 1536 free-dim elements per partition

# --- schedule configuration (tunable) ---
A_COLS = 821               # DVE column share; Pool gets F - A_COLS
GROUPS_V = [1] * 16        # steps per DMA slab, DVE chunk
GROUPS_G = [1] * 16        # steps per DMA slab, Pool chunk
POOL_TAIL = 2              # last N Pool step slabs go out Pool's own queue

# cost constants (CoreSim v1 model, TRN2) for the static schedule
_DVE_TT = 0.5208
_DVE_TS = 0.2604
_DVE_OVH = 60.0
_POOL_OP = 0.8333
_DMA_NSB = 0.0030117
_DMA_MIN = 500.0
_SEM = 100.0

_prog_cache: dict = {}


def _schedule():
    """Static beam-search schedule over the calibrated cost model.
    Returns slabs in emission order: (chunk, step_off, gsize, queue,
    dram_off, start_ns); chunk 0 = DVE, 1 = Pool. Pool's queue is only
    available once its chain ends (its DMAs are emitted at the end of
    its instruction stream)."""
    a, b = A_COLS, F - A_COLS
    # slab ready times (measured: DVE starts at input-DMA end + 1717;
    # Pool sees its own queue's input at transfer end with no latency)
    t = 200 + max(P * a * 2 * _DMA_NSB, _DMA_MIN) + 1716.7
    t += 2 * (_DVE_TT * a + _DVE_OVH) + 2 * (_DVE_TS * a + _DVE_OVH)
    v_ready = []
    o = 0
    for g in GROUPS_V:
        t += g * (_DVE_TT * a + _DVE_OVH)
        v_ready.append((t, 0, o, g, P * g * a * 2))
        o += g
    t = 100 + max(P * b * 2 * _DMA_NSB, _DMA_MIN) + 3 * _POOL_OP * b
    g_ready = []
    o = 0
    for g in GROUPS_G:
        t += g * _POOL_OP * b
        g_ready.append((t, 1, o, g, P * g * b * 2))
        o += g
    pool_free = t
    slabs = sorted(v_ready + g_ready)
    # beam search over (sp_end, act_end, pool_end)
    sp0 = 200 + max(P * a * 2 * _DMA_NSB, _DMA_MIN)
    states = {(sp0, 200.0, pool_free): ()}
    for r, c, o, g, nb in slabs:
        dur = max(nb * _DMA_NSB, _DMA_MIN)
        new = {}
        for ends, hist in states.items():
            for qi in range(3):
                end = max(ends[qi], r + _SEM) + dur
                ns = tuple(end if i == qi else v for i, v in enumerate(ends))
                if ns not in new:
                    new[ns] = hist + (qi,)
        states = dict(sorted(new.items(), key=lambda kv: max(kv[0]))[:600])
    best = min(states.items(), key=lambda kv: max(kv[0]))
    qnames = ('s', 'a', 'g')
    # recompute per-queue start times of the winning assignment
    ends = {'s': sp0, 'a': 200.0, 'g': pool_free}
    assigned = []
    for (r, c, o, g, nb), qi in zip(slabs, best[1]):
        q = qnames[qi]
        dur = max(nb * _DMA_NSB, _DMA_MIN)
        t0 = max(ends[q], r + _SEM)
        ends[q] = t0 + dur
        assigned.append((c, o, g, nb, q, t0))
    res = []
    off = 0
    for c, o, g, nb, q, t0 in assigned:
        res.append((c, o, g, q, off, t0))
        off += nb // 2
    assert off == S * P * F
    return res


def _build_program(w: float, c0: float | None):
    """c0: uniform coefficient, or None -> per-element coef input."""
    nc = bacc.Bacc("TRN2", target_bir_lowering=False, debug=False)
    f16 = mybir.dt.float16
    Alu = mybir.AluOpType

    x_in = nc.declare_dram_parameter("x0", [P, F], f16, isOutput=False)
    coef_in = None
    if c0 is None:
        coef_in = nc.declare_dram_parameter("coef", [P, F], f16, isOutput=False)
    out = nc.declare_dram_parameter("out", [S * P * F], f16, isOutput=True)

    slabs = _schedule()
    a, b = A_COLS, F - A_COLS
    cols = [(0, a), (a, b)]            # (col0, fc) per chunk
    engs = ['v', 'g']
    cs = STEP_SIZE * (c0 if c0 is not None else 1.0)
    Ac = float(BETA * cs)
    Bc = float(w * cs)

    with ExitStack() as ctx:
        tc = ctx.enter_context(tile.TileContext(nc))
        cpool = ctx.enter_context(tc.tile_pool(name="const", bufs=1))
        gpool = ctx.enter_context(tc.tile_pool(name="groups", bufs=1))

        q_map = {'s': lambda: nc.sync, 'a': lambda: nc.scalar,
                 'g': lambda: nc.gpsimd}

        def eng_of(c):
            return nc.vector if engs[c] == 'v' else nc.gpsimd

        # inputs: DVE share via SP, Pool share via Pool's own SWDGE queue
        x0 = []
        for c, (c0_, fc) in enumerate(cols):
            t = cpool.tile([P, fc], f16, name=f"x0_{c}", tag=f"x0_{c}")
            q_map['s' if engs[c] == 'v' else 'g']().dma_start(
                t[:], x_in[:, c0_: c0_ + fc])
            x0.append(t)
        coef = []
        if c0 is None:
            for c, (c0_, fc) in enumerate(cols):
                t = cpool.tile([P, fc], f16, name=f"cf_{c}", tag=f"cf_{c}")
                q_map['a']().dma_start(t[:], coef_in[:, c0_: c0_ + fc])
                coef.append(t)

        dm = [None, None]

        def prologue(c):
            fc = cols[c][1]
            eng = eng_of(c)
            t2 = cpool.tile([P, fc], f16, name=f"t2_{c}", tag=f"t2_{c}")
            eng.tensor_tensor(t2[:], x0[c][:], x0[c][:], Alu.mult)
            d = cpool.tile([P, fc], f16, name=f"d_{c}", tag=f"d_{c}")
            if engs[c] == 'v':
                # delta = (1/4 - x^2/16) * (A*x + B), exact product form
                sc = cpool.tile([P, fc], f16, name=f"s_{c}", tag=f"s_{c}")
                eng.tensor_scalar(sc[:], t2[:], -1.0 / 16.0, 0.25,
                                  Alu.mult, Alu.add)
                uc = cpool.tile([P, fc], f16, name=f"u_{c}", tag=f"u_{c}")
                eng.tensor_scalar(uc[:], x0[c][:], Ac, Bc, Alu.mult, Alu.add)
                if c0 is None:
                    eng.tensor_tensor(uc[:], uc[:], coef[c][:], Alu.mult)
                eng.tensor_tensor(d[:], sc[:], uc[:], Alu.mult)
            else:
                # delta ~= B/4 + (A/4)x - (B/16)x^2 (x^3 term is ~1% of a
                # delta that is itself 1e-9 against fp16's 6e-5 resolution)
                rc = cpool.tile([P, fc], f16, name=f"r_{c}", tag=f"r_{c}")
                eng.tensor_scalar(rc[:], t2[:], -Bc / 16.0, Bc / 4.0,
                                  Alu.mult, Alu.add)
                eng.scalar_tensor_tensor(d[:], x0[c][:], Ac / 4.0, rc[:],
                                         Alu.mult, Alu.add)
                if c0 is None:
                    eng.tensor_tensor(d[:], d[:], coef[c][:], Alu.mult)
            dm[c] = d

        # per-step tiles (groups of 1): st[c][t-1] holds step t
        st = [
            [gpool.tile([P, fc], f16, name=f"st_{c}_{t}", tag=f"st_{c}_{t}")
             for t in range(S)]
            for c, (_, fc) in enumerate(cols)
        ]

        def step_ap(c, t):
            return x0[c][:] if t == 0 else st[c][t - 1][:]

        # emit compute ops and out-DMAs merged in planned-time order so the
        # tile scheduler sees (and keeps) the intended per-queue ordering
        events = []
        a_, b_ = A_COLS, F - A_COLS
        tv = 200 + max(P * a_ * 2 * _DMA_NSB, _DMA_MIN) + 1716.7
        tg = 100 + max(P * b_ * 2 * _DMA_NSB, _DMA_MIN)
        events.append((tv, 0, ('prol', 0)))
        events.append((tg, 0, ('prol', 1)))
        tv += 2 * (_DVE_TT * a_ + _DVE_OVH) + 2 * (_DVE_TS * a_ + _DVE_OVH)
        tg += 3 * _POOL_OP * b_
        for t in range(1, S + 1):
            events.append((tv, t, ('step', 0, t)))
            events.append((tg, t, ('step', 1, t)))
            tv += _DVE_TT * a_ + _DVE_OVH
            tg += _POOL_OP * b_
        for c, o, g, q, off, t0 in slabs:
            events.append((t0, 100 + o, ('dma', c, o, g, q, off)))
        events.sort(key=lambda e: (e[0], e[1]))
        for _, _, ev in events:
            if ev[0] == 'prol':
                prologue(ev[1])
            elif ev[0] == 'step':
                c, t = ev[1], ev[2]
                eng_of(c).tensor_tensor(step_ap(c, t), step_ap(c, t - 1),
                                        dm[c][:], Alu.subtract)
            else:
                _, c, o, g, q, off = ev
                fc = cols[c][1]
                dst = out[off: off + P * fc].rearrange("(p x) -> p x", p=P)
                q_map[q]().dma_start(dst, st[c][o][:])

    nc.compile()
    return nc


def kernel(features, predictions_init, W_feat, w_prob, b, attention_mask):
    preds = np.asarray(predictions_init, dtype=np.float32)
    mask = attention_mask.astype(np.float32)
    horses = mask.sum(axis=-1)                       # [B]
    c = (mask * mask) / (horses[:, None] * (V * B))  # [B,H]
    w = float(np.asarray(w_prob).reshape(-1)[0])

    c0 = float(c.flat[0])
    uniform = bool(np.all(c == c0))

    key = (w, c0 if uniform else None)
    if key not in _prog_cache:
        _prog_cache[key] = _build_program(w, c0 if uniform else None)
    nc = _prog_cache[key]

    in_maps = []
    for core in range(NCORES):
        shard = preds[core * VSH: (core + 1) * VSH].reshape(P, F)
        m = {"x0": shard.astype(np.float16)}
        if not uniform:
            ctile = np.broadcast_to(
                (c * STEP_SIZE)[None], (VSH, B, H)).reshape(P, F)
            m["coef"] = ctile.astype(np.float16)
        in_maps.append(m)

    res = run_bass_kernel_spmd(nc, in_maps, core_ids=list(range(NCORES)))

    slabs = _schedule()
    a = A_COLS
    cstart = [0, a]
    cwidth = [a, F - a]
    outs = []
    for r in res.results:
        arr = np.asarray(r["out"])
        result = np.empty((S, P, F), dtype=np.float16)
        for c, o, g, q, off, t0 in slabs:
            fc = cwidth[c]
            block = arr[off: off + P * g * fc].reshape(P, g, fc)
            result[o: o + g, :, cstart[c]: cstart[c] + fc] = (
                block.transpose(1, 0, 2)
            )
        outs.append(result.reshape(S, VSH, B, H))
    full = np.concatenate(outs, axis=1)              # [S, V, B, H]
    return np.ascontiguousarray(full[..., None].astype(np.float32))


# revision 12
# speedup vs baseline: 1.0486x; 1.0486x over previous
"""Trainium2 Bass kernel for nn_MCMCSampler.

Math: the energy gradient w.r.t. preds is purely elementwise (the feature
einsum is constant w.r.t. preds so it drops out of jax.grad):

    p     = sigmoid(x)
    grad  = c * p(1-p) * (w + beta*L),   c[b,h] = mask[b,h]/(horses[b]*V*B)
    x    <- x - STEP*grad*mask

where L = dentropy/dp collapses to x (logit o sigmoid identity, eps terms
cancel at O(eps^2)). The update is ~8e-10 per step against x ~ 0.1, so
delta is constant across the 16 steps to ~1e-16: compute delta once from
x0 and chain x_t = x_{t-1} - delta.

Precision: fp16 end to end. sigmoid'(x) = p(1-p) is evaluated via its
Taylor series 1/4 - x^2/16 (|x| <= ~0.6 here; the series error lands on
a delta that only moves the output at the 1e-9 level, far below fp16
resolution). fp16 halves DMA traffic and unlocks the DVE 2x_1p/4x_2p
perf modes. Output error vs the fp32 reference is the fp16 quantization
floor, ~2e-4 rel.

Sharding: data-parallel over V (64 variants / 8 cores), no cross-core
communication. Per-core output: [16, 128*1536] fp16 = 6.3 MB.

Schedule (CoreSim v1 cost model): a DMA occupies its issuing engine
queue (SP / ACT HWDGE, Pool SWDGE) for bytes/332GB/s; queues run
concurrently; a DMA's data is visible to the *issuing* engine's later
instructions at transfer end but to other engines only ~1.7us later
(sem path). Exploits:
  - exactly ONE column-chunk per engine, so the chain's data deps pin
    the per-engine instruction order (no compile-time round-robin
    interleave -> no head-of-line blocking);
  - Pool loads its own x0 share through its SWDGE queue and starts
    chaining ~0.6us in, vs 2.4us for the DVE share (SP load + sem);
  - per-step DMA slabs assigned to SP/ACT (and Pool once its chains
    end) by a small beam search over the measured cost constants.
"""

import numpy as np
from contextlib import ExitStack

import concourse.bass as bass
from concourse import bacc
import concourse.mybir as mybir
import concourse.tile as tile
from concourse.bass_utils import run_bass_kernel_spmd

NCORES = 8
V, B, H = 64, 1024, 24
S = 16
STEP_SIZE = 0.1
BETA = 0.1
VSH = V // NCORES          # 8 variants per core
N = VSH * B * H            # 196608 elements per core
P = 128                    # SBUF partitions
F = N // P                 # 1536 free-dim elements per partition

# --- schedule configuration (tunable) ---
A_COLS = 821               # DVE column share; Pool gets F - A_COLS
GROUPS_V = [1] * 16        # steps per DMA slab, DVE chunk
GROUPS_G = [1] * 16        # steps per DMA slab, Pool chunk
POOL_TAIL = 2              # last N Pool step slabs go out Pool's own queue

# cost constants (CoreSim v1 model, TRN2) for the static schedule
_DVE_TT = 0.5208
_DVE_TS = 0.2604
_DVE_OVH = 60.0
_POOL_OP = 0.8333
_DMA_NSB = 0.0030117
_DMA_MIN = 500.0
_SEM = 100.0

_prog_cache: dict = {}


def _schedule():
    """Static beam-search schedule over the calibrated cost model.
    Returns slabs in emission order: (chunk, step_off, gsize, queue,
    dram_off, start_ns); chunk 0 = DVE, 1 = Pool. The last POOL_TAIL
    pool slabs ride Pool's own queue (emitted at the end of its stream,
    after its chains); the rest beam-search over SP/ACT."""
    a, b = A_COLS, F - A_COLS
    # slab ready times (measured: DVE starts at input-DMA end + 1717,
    # input split in two parallel half-loads on SP+ACT; Pool sees its
    # own queue's input at transfer end with no latency)
    in_v = max(P * a * _DMA_NSB, _DMA_MIN)       # half-width loads x2
    t = 200 + in_v + 1716.7
    t += 2 * (_DVE_TT * a + _DVE_OVH) + 2 * (_DVE_TS * a + _DVE_OVH)
    v_ready = []
    o = 0
    for g in GROUPS_V:
        t += g * (_DVE_TT * a + _DVE_OVH)
        v_ready.append((t, 0, o, g, P * g * a * 2))
        o += g
    t = 100 + max(P * b * 2 * _DMA_NSB, _DMA_MIN) + 3 * _POOL_OP * b
    g_ready = []
    o = 0
    for g in GROUPS_G:
        t += g * _POOL_OP * b
        g_ready.append((t, 1, o, g, P * g * b * 2))
        o += g
    pool_free = t
    tail = g_ready[len(GROUPS_G) - POOL_TAIL:] if POOL_TAIL else []
    head = g_ready[: len(GROUPS_G) - POOL_TAIL] if POOL_TAIL else g_ready
    slabs = sorted(v_ready + head)
    q0 = 200 + in_v
    states = {(q0, q0): ()}
    for r, c, o, g, nb in slabs:
        dur = max(nb * _DMA_NSB, _DMA_MIN)
        new = {}
        for ends, hist in states.items():
            for qi in range(2):
                end = max(ends[qi], r + _SEM) + dur
                ns = tuple(end if i == qi else v for i, v in enumerate(ends))
                if ns not in new:
                    new[ns] = hist + (qi,)
        states = dict(sorted(new.items(), key=lambda kv: max(kv[0]))[:600])
    best = min(states.items(), key=lambda kv: max(kv[0]))
    qnames = ('s', 'a')
    ends = {'s': q0, 'a': q0, 'g': pool_free}
    assigned = []
    for (r, c, o, g, nb), qi in zip(slabs, best[1]):
        q = qnames[qi]
        dur = max(nb * _DMA_NSB, _DMA_MIN)
        t0 = max(ends[q], r + _SEM)
        ends[q] = t0 + dur
        assigned.append((c, o, g, nb, q, t0))
    for r, c, o, g, nb in tail:
        dur = max(nb * _DMA_NSB, _DMA_MIN)
        t0 = max(ends['g'], r + _SEM)
        ends['g'] = t0 + dur
        assigned.append((c, o, g, nb, 'g', t0))
    res = []
    off = 0
    for c, o, g, nb, q, t0 in assigned:
        res.append((c, o, g, q, off, t0))
        off += nb // 2
    assert off == S * P * F
    return res


def _build_program(w: float, c0: float | None):
    """c0: uniform coefficient, or None -> per-element coef input."""
    nc = bacc.Bacc("TRN2", target_bir_lowering=False, debug=False)
    f16 = mybir.dt.float16
    Alu = mybir.AluOpType

    x_in = nc.declare_dram_parameter("x0", [P, F], f16, isOutput=False)
    coef_in = None
    if c0 is None:
        coef_in = nc.declare_dram_parameter("coef", [P, F], f16, isOutput=False)
    out = nc.declare_dram_parameter("out", [S * P * F], f16, isOutput=True)

    slabs = _schedule()
    a, b = A_COLS, F - A_COLS
    cols = [(0, a), (a, b)]            # (col0, fc) per chunk
    engs = ['v', 'g']
    cs = STEP_SIZE * (c0 if c0 is not None else 1.0)
    Ac = float(BETA * cs)
    Bc = float(w * cs)

    with ExitStack() as ctx:
        tc = ctx.enter_context(tile.TileContext(nc))
        cpool = ctx.enter_context(tc.tile_pool(name="const", bufs=1))
        gpool = ctx.enter_context(tc.tile_pool(name="groups", bufs=1))

        q_map = {'s': lambda: nc.sync, 'a': lambda: nc.scalar,
                 'g': lambda: nc.gpsimd}

        def eng_of(c):
            return nc.vector if engs[c] == 'v' else nc.gpsimd

        # inputs: DVE share split across SP+ACT in parallel half-loads,
        # Pool share via Pool's own SWDGE queue (visible at transfer end)
        x0 = []
        for c, (c0_, fc) in enumerate(cols):
            t = cpool.tile([P, fc], f16, name=f"x0_{c}", tag=f"x0_{c}")
            if engs[c] == 'v':
                h = fc // 2
                nc.sync.dma_start(t[:, :h], x_in[:, c0_: c0_ + h])
                nc.scalar.dma_start(t[:, h:], x_in[:, c0_ + h: c0_ + fc])
            else:
                nc.gpsimd.dma_start(t[:], x_in[:, c0_: c0_ + fc])
            x0.append(t)
        coef = []
        if c0 is None:
            for c, (c0_, fc) in enumerate(cols):
                t = cpool.tile([P, fc], f16, name=f"cf_{c}", tag=f"cf_{c}")
                q_map['a']().dma_start(t[:], coef_in[:, c0_: c0_ + fc])
                coef.append(t)

        dm = [None, None]

        def prologue(c):
            fc = cols[c][1]
            eng = eng_of(c)
            t2 = cpool.tile([P, fc], f16, name=f"t2_{c}", tag=f"t2_{c}")
            eng.tensor_tensor(t2[:], x0[c][:], x0[c][:], Alu.mult)
            d = cpool.tile([P, fc], f16, name=f"d_{c}", tag=f"d_{c}")
            if engs[c] == 'v':
                # delta = (1/4 - x^2/16) * (A*x + B), exact product form
                sc = cpool.tile([P, fc], f16, name=f"s_{c}", tag=f"s_{c}")
                eng.tensor_scalar(sc[:], t2[:], -1.0 / 16.0, 0.25,
                                  Alu.mult, Alu.add)
                uc = cpool.tile([P, fc], f16, name=f"u_{c}", tag=f"u_{c}")
                eng.tensor_scalar(uc[:], x0[c][:], Ac, Bc, Alu.mult, Alu.add)
                if c0 is None:
                    eng.tensor_tensor(uc[:], uc[:], coef[c][:], Alu.mult)
                eng.tensor_tensor(d[:], sc[:], uc[:], Alu.mult)
            else:
                # delta ~= B/4 + (A/4)x - (B/16)x^2 (x^3 term is ~1% of a
                # delta that is itself 1e-9 against fp16's 6e-5 resolution)
                rc = cpool.tile([P, fc], f16, name=f"r_{c}", tag=f"r_{c}")
                eng.tensor_scalar(rc[:], t2[:], -Bc / 16.0, Bc / 4.0,
                                  Alu.mult, Alu.add)
                eng.scalar_tensor_tensor(d[:], x0[c][:], Ac / 4.0, rc[:],
                                         Alu.mult, Alu.add)
                if c0 is None:
                    eng.tensor_tensor(d[:], d[:], coef[c][:], Alu.mult)
            dm[c] = d

        # per-step tiles (groups of 1): st[c][t-1] holds step t
        st = [
            [gpool.tile([P, fc], f16, name=f"st_{c}_{t}", tag=f"st_{c}_{t}")
             for t in range(S)]
            for c, (_, fc) in enumerate(cols)
        ]

        def step_ap(c, t):
            return x0[c][:] if t == 0 else st[c][t - 1][:]

        # emit compute ops and out-DMAs merged in planned-time order so the
        # tile scheduler sees (and keeps) the intended per-queue ordering
        events = []
        a_, b_ = A_COLS, F - A_COLS
        tv = 200 + max(P * a_ * 2 * _DMA_NSB, _DMA_MIN) + 1716.7
        tg = 100 + max(P * b_ * 2 * _DMA_NSB, _DMA_MIN)
        events.append((tv, 0, ('prol', 0)))
        events.append((tg, 0, ('prol', 1)))
        tv += 2 * (_DVE_TT * a_ + _DVE_OVH) + 2 * (_DVE_TS * a_ + _DVE_OVH)
        tg += 3 * _POOL_OP * b_
        for t in range(1, S + 1):
            events.append((tv, t, ('step', 0, t)))
            events.append((tg, t, ('step', 1, t)))
            tv += _DVE_TT * a_ + _DVE_OVH
            tg += _POOL_OP * b_
        for c, o, g, q, off, t0 in slabs:
            events.append((t0, 100 + o, ('dma', c, o, g, q, off)))
        events.sort(key=lambda e: (e[0], e[1]))
        for _, _, ev in events:
            if ev[0] == 'prol':
                prologue(ev[1])
            elif ev[0] == 'step':
                c, t = ev[1], ev[2]
                eng_of(c).tensor_tensor(step_ap(c, t), step_ap(c, t - 1),
                                        dm[c][:], Alu.subtract)
            else:
                _, c, o, g, q, off = ev
                fc = cols[c][1]
                dst = out[off: off + P * fc].rearrange("(p x) -> p x", p=P)
                q_map[q]().dma_start(dst, st[c][o][:])

    nc.compile()
    return nc


def kernel(features, predictions_init, W_feat, w_prob, b, attention_mask):
    preds = np.asarray(predictions_init, dtype=np.float32)
    mask = attention_mask.astype(np.float32)
    horses = mask.sum(axis=-1)                       # [B]
    c = (mask * mask) / (horses[:, None] * (V * B))  # [B,H]
    w = float(np.asarray(w_prob).reshape(-1)[0])

    c0 = float(c.flat[0])
    uniform = bool(np.all(c == c0))

    key = (w, c0 if uniform else None)
    if key not in _prog_cache:
        _prog_cache[key] = _build_program(w, c0 if uniform else None)
    nc = _prog_cache[key]

    in_maps = []
    for core in range(NCORES):
        shard = preds[core * VSH: (core + 1) * VSH].reshape(P, F)
        m = {"x0": shard.astype(np.float16)}
        if not uniform:
            ctile = np.broadcast_to(
                (c * STEP_SIZE)[None], (VSH, B, H)).reshape(P, F)
            m["coef"] = ctile.astype(np.float16)
        in_maps.append(m)

    res = run_bass_kernel_spmd(nc, in_maps, core_ids=list(range(NCORES)))

    slabs = _schedule()
    a = A_COLS
    cstart = [0, a]
    cwidth = [a, F - a]
    outs = []
    for r in res.results:
        arr = np.asarray(r["out"])
        result = np.empty((S, P, F), dtype=np.float16)
        for c, o, g, q, off, t0 in slabs:
            fc = cwidth[c]
            block = arr[off: off + P * g * fc].reshape(P, g, fc)
            result[o: o + g, :, cstart[c]: cstart[c] + fc] = (
                block.transpose(1, 0, 2)
            )
        outs.append(result.reshape(S, VSH, B, H))
    full = np.concatenate(outs, axis=1)              # [S, V, B, H]
    return np.ascontiguousarray(full[..., None].astype(np.float32))


# revision 20
# speedup vs baseline: 1.0998x; 1.0489x over previous
"""Trainium2 Bass kernel for nn_MCMCSampler.

Math: the energy gradient w.r.t. preds is purely elementwise (the feature
einsum is constant w.r.t. preds so it drops out of jax.grad):

    p     = sigmoid(x)
    grad  = c * p(1-p) * (w + beta*L),   c[b,h] = mask[b,h]/(horses[b]*V*B)
    x    <- x - STEP*grad*mask

where L = dentropy/dp collapses to x (logit o sigmoid identity, eps terms
cancel at O(eps^2)). The update is ~8e-10 per step against x ~ 0.1, so
delta is constant across the 16 steps to ~1e-16: compute delta once from
x0 and chain x_t = x_{t-1} - delta.

Precision: fp16 end to end. sigmoid'(x) = p(1-p) is evaluated via its
Taylor series 1/4 - x^2/16 (|x| <= ~0.6 here; the series error lands on
a delta that only moves the output at the 1e-9 level, far below fp16
resolution). fp16 halves DMA traffic and unlocks the DVE 2x_1p/4x_2p
perf modes. Output error vs the fp32 reference is the fp16 quantization
floor, ~2e-4 rel.

Sharding: data-parallel over V (64 variants / 8 cores), no cross-core
communication. Per-core output: [16, 128*1536] fp16 = 6.3 MB.

Schedule (CoreSim v1 cost model): a DMA occupies its issuing engine
queue (SP / ACT HWDGE, Pool SWDGE) for bytes/332GB/s; queues run
concurrently; a DMA's data is visible to the *issuing* engine's later
instructions at transfer end but to other engines only ~1.7us later
(sem path). Exploits:
  - exactly ONE column-chunk per engine, so the chain's data deps pin
    the per-engine instruction order (no compile-time round-robin
    interleave -> no head-of-line blocking);
  - Pool loads its own x0 share through its SWDGE queue and starts
    chaining ~0.6us in, vs 2.4us for the DVE share (SP load + sem);
  - per-step DMA slabs assigned to SP/ACT (and Pool once its chains
    end) by a small beam search over the measured cost constants.
"""

import numpy as np
from contextlib import ExitStack

import concourse.bass as bass
from concourse import bacc
import concourse.mybir as mybir
import concourse.tile as tile
from concourse.bass_utils import run_bass_kernel_spmd

NCORES = 8
V, B, H = 64, 1024, 24
S = 16
STEP_SIZE = 0.1
BETA = 0.1
VSH = V // NCORES          # 8 variants per core
N = VSH * B * H            # 196608 elements per core
P = 128                    # SBUF partitions
F = N // P                 # 1536 free-dim elements per partition

# --- schedule configuration (tunable) ---
A_COLS = 865               # DVE column share; Pool gets F - A_COLS
GROUPS_V = [1] * 16        # steps per DMA slab, DVE chunk
GROUPS_G = [1] * 16        # steps per DMA slab, Pool chunk
POOL_TAIL = 4              # pool-queue carries the last N pool-computed slabs
STEAL = 2                  # DVE computes the last STEAL steps of Pool's chunk

# cost constants (CoreSim v1 model, TRN2) for the static schedule
_DVE_TT = 0.5208
_DVE_TS = 0.2604
_DVE_OVH = 60.0
_POOL_OP = 0.8333
_DMA_NSB = 0.0030117
_DMA_MIN = 500.0
_SEM = 100.0

_prog_cache: dict = {}


def _schedule():
    """Static beam-search schedule over the calibrated cost model.
    Returns slabs in emission order: (chunk, step_off, gsize, queue,
    dram_off, start_ns); chunk 0 = DVE, 1 = Pool. The last POOL_TAIL
    pool slabs ride Pool's own queue (emitted at the end of its stream,
    after its chains); the rest beam-search over SP/ACT."""
    a, b = A_COLS, F - A_COLS
    # slab ready times (measured: DVE starts at input-DMA end + 1717,
    # input split in two parallel half-loads on SP+ACT; Pool sees its
    # own queue's input at transfer end with no latency)
    in_v = max(P * a * _DMA_NSB, _DMA_MIN)       # half-width loads x2
    t = 200 + in_v + 1716.7
    t += 2 * (_DVE_TT * a + _DVE_OVH) + 2 * (_DVE_TS * a + _DVE_OVH)
    v_ready = []
    o = 0
    for g in GROUPS_V:
        t += g * (_DVE_TT * a + _DVE_OVH)
        v_ready.append((t, 0, o, g, P * g * a * 2))
        o += g
    tv_end = t
    t = 100 + max(P * b * 2 * _DMA_NSB, _DMA_MIN) + 4 * _POOL_OP * b
    g_ready = []
    o = 0
    for g in GROUPS_G[: S - STEAL]:
        t += g * _POOL_OP * b
        g_ready.append((t, 1, o, g, P * g * b * 2))
        o += g
    pool_free = t
    # stolen steps run on DVE after its own chain
    t = max(tv_end, t + _SEM)
    for g in GROUPS_G[S - STEAL:]:
        t += g * (_DVE_TT * b + _DVE_OVH)
        g_ready.append((t, 1, o, g, P * g * b * 2))
        o += g
    ncomp = len(GROUPS_G) - STEAL
    tail = g_ready[ncomp - POOL_TAIL: ncomp] if POOL_TAIL else []
    head = g_ready[: ncomp - POOL_TAIL] + g_ready[ncomp:] if POOL_TAIL else g_ready
    slabs = sorted(v_ready + head)
    q0 = 200 + in_v
    states = {(q0, q0): ()}
    for r, c, o, g, nb in slabs:
        dur = max(nb * _DMA_NSB, _DMA_MIN)
        new = {}
        for ends, hist in states.items():
            for qi in range(2):
                end = max(ends[qi], r + _SEM) + dur
                ns = tuple(end if i == qi else v for i, v in enumerate(ends))
                if ns not in new:
                    new[ns] = hist + (qi,)
        states = dict(sorted(new.items(), key=lambda kv: max(kv[0]))[:600])
    best = min(states.items(), key=lambda kv: max(kv[0]))
    qnames = ('s', 'a')
    ends = {'s': q0, 'a': q0, 'g': pool_free}
    assigned = []
    for (r, c, o, g, nb), qi in zip(slabs, best[1]):
        q = qnames[qi]
        dur = max(nb * _DMA_NSB, _DMA_MIN)
        t0 = max(ends[q], r + _SEM)
        ends[q] = t0 + dur
        assigned.append((c, o, g, nb, q, t0))
    for r, c, o, g, nb in tail:
        dur = max(nb * _DMA_NSB, _DMA_MIN)
        t0 = max(ends['g'], r + _SEM)
        ends['g'] = t0 + dur
        assigned.append((c, o, g, nb, 'g', t0))
    res = []
    off = 0
    for c, o, g, nb, q, t0 in assigned:
        res.append((c, o, g, q, off, t0))
        off += nb // 2
    assert off == S * P * F
    return res


def _build_program(w: float, c0: float | None):
    """c0: uniform coefficient, or None -> per-element coef input."""
    nc = bacc.Bacc("TRN2", target_bir_lowering=False, debug=False)
    f16 = mybir.dt.float16
    Alu = mybir.AluOpType

    x_in = nc.declare_dram_parameter("x0", [P, F], f16, isOutput=False)
    coef_in = None
    if c0 is None:
        coef_in = nc.declare_dram_parameter("coef", [P, F], f16, isOutput=False)
    out = nc.declare_dram_parameter("out", [S * P * F], f16, isOutput=True)

    slabs = _schedule()
    a, b = A_COLS, F - A_COLS
    cols = [(0, a), (a, b)]            # (col0, fc) per chunk
    engs = ['v', 'g']
    cs = STEP_SIZE * (c0 if c0 is not None else 1.0)
    Ac = float(BETA * cs)
    Bc = float(w * cs)

    with ExitStack() as ctx:
        tc = ctx.enter_context(tile.TileContext(nc))
        cpool = ctx.enter_context(tc.tile_pool(name="const", bufs=1))
        gpool = ctx.enter_context(tc.tile_pool(name="groups", bufs=1))

        q_map = {'s': lambda: nc.sync, 'a': lambda: nc.scalar,
                 'g': lambda: nc.gpsimd}

        def eng_of(c):
            return nc.vector if engs[c] == 'v' else nc.gpsimd

        # inputs: DVE share split across SP+ACT in parallel half-loads,
        # Pool share via Pool's own SWDGE queue (visible at transfer end)
        x0 = []
        for c, (c0_, fc) in enumerate(cols):
            t = cpool.tile([P, fc], f16, name=f"x0_{c}", tag=f"x0_{c}")
            if engs[c] == 'v':
                h = fc // 2
                nc.sync.dma_start(t[:, :h], x_in[:, c0_: c0_ + h])
                nc.scalar.dma_start(t[:, h:], x_in[:, c0_ + h: c0_ + fc])
            else:
                nc.gpsimd.dma_start(t[:], x_in[:, c0_: c0_ + fc])
            x0.append(t)
        coef = []
        if c0 is None:
            for c, (c0_, fc) in enumerate(cols):
                t = cpool.tile([P, fc], f16, name=f"cf_{c}", tag=f"cf_{c}")
                q_map['a']().dma_start(t[:], coef_in[:, c0_: c0_ + fc])
                coef.append(t)

        dm = [None, None]

        def prologue(c):
            fc = cols[c][1]
            eng = eng_of(c)
            t2 = cpool.tile([P, fc], f16, name=f"t2_{c}", tag=f"t2_{c}")
            eng.tensor_tensor(t2[:], x0[c][:], x0[c][:], Alu.mult)
            d = cpool.tile([P, fc], f16, name=f"d_{c}", tag=f"d_{c}")
            # delta = (1/4 - x^2/16) * (A*x + B): tensor_scalar +
            # tensor_tensor only (scalar_tensor_tensor is not legal on
            # Pool in the hardware ISA)
            sc = cpool.tile([P, fc], f16, name=f"s_{c}", tag=f"s_{c}")
            eng.tensor_scalar(sc[:], t2[:], -1.0 / 16.0, 0.25,
                              Alu.mult, Alu.add)
            uc = cpool.tile([P, fc], f16, name=f"u_{c}", tag=f"u_{c}")
            eng.tensor_scalar(uc[:], x0[c][:], Ac, Bc, Alu.mult, Alu.add)
            if c0 is None:
                eng.tensor_tensor(uc[:], uc[:], coef[c][:], Alu.mult)
            eng.tensor_tensor(d[:], sc[:], uc[:], Alu.mult)
            dm[c] = d

        # per-step tiles (groups of 1): st[c][t-1] holds step t
        st = [
            [gpool.tile([P, fc], f16, name=f"st_{c}_{t}", tag=f"st_{c}_{t}")
             for t in range(S)]
            for c, (_, fc) in enumerate(cols)
        ]

        def step_ap(c, t):
            return x0[c][:] if t == 0 else st[c][t - 1][:]

        # emit compute ops and out-DMAs merged in planned-time order so the
        # tile scheduler sees (and keeps) the intended per-queue ordering
        events = []
        a_, b_ = A_COLS, F - A_COLS
        tv = 200 + max(P * a_ * 2 * _DMA_NSB, _DMA_MIN) + 1716.7
        tg = 100 + max(P * b_ * 2 * _DMA_NSB, _DMA_MIN)
        events.append((tv, 0, ('prol', 0)))
        events.append((tg, 0, ('prol', 1)))
        tv += 2 * (_DVE_TT * a_ + _DVE_OVH) + 2 * (_DVE_TS * a_ + _DVE_OVH)
        tg += 4 * _POOL_OP * b_
        for t in range(1, S + 1):
            events.append((tv, t, ('step', 0, t)))
            tv += _DVE_TT * a_ + _DVE_OVH
            if t <= S - STEAL:
                events.append((tg, t, ('step', 1, t)))
                tg += _POOL_OP * b_
        for t in range(S - STEAL + 1, S + 1):
            events.append((tv, t, ('step', 1, t)))
            tv += _DVE_TT * b_ + _DVE_OVH
        for c, o, g, q, off, t0 in slabs:
            events.append((t0, 100 + o, ('dma', c, o, g, q, off)))
        events.sort(key=lambda e: (e[0], e[1]))
        for _, _, ev in events:
            if ev[0] == 'prol':
                prologue(ev[1])
            elif ev[0] == 'step':
                c, t = ev[1], ev[2]
                eng = eng_of(c)
                if c == 1 and t > S - STEAL:
                    eng = nc.vector
                eng.tensor_tensor(step_ap(c, t), step_ap(c, t - 1),
                                  dm[c][:], Alu.subtract)
            else:
                _, c, o, g, q, off = ev
                fc = cols[c][1]
                dst = out[off: off + P * fc].rearrange("(p x) -> p x", p=P)
                q_map[q]().dma_start(dst, st[c][o][:])

    nc.compile()
    return nc


def kernel(features, predictions_init, W_feat, w_prob, b, attention_mask):
    preds = np.asarray(predictions_init, dtype=np.float32)
    mask = attention_mask.astype(np.float32)
    horses = mask.sum(axis=-1)                       # [B]
    c = (mask * mask) / (horses[:, None] * (V * B))  # [B,H]
    w = float(np.asarray(w_prob).reshape(-1)[0])

    c0 = float(c.flat[0])
    uniform = bool(np.all(c == c0))

    key = (w, c0 if uniform else None)
    if key not in _prog_cache:
        _prog_cache[key] = _build_program(w, c0 if uniform else None)
    nc = _prog_cache[key]

    in_maps = []
    for core in range(NCORES):
        shard = preds[core * VSH: (core + 1) * VSH].reshape(P, F)
        m = {"x0": shard.astype(np.float16)}
        if not uniform:
            # Ac/Bc already fold in STEP_SIZE (cs = STEP_SIZE * 1.0), so the
            # coef input is the bare per-element c
            ctile = np.broadcast_to(c[None], (VSH, B, H)).reshape(P, F)
            m["coef"] = ctile.astype(np.float16)
        in_maps.append(m)

    res = run_bass_kernel_spmd(nc, in_maps, core_ids=list(range(NCORES)))

    slabs = _schedule()
    a = A_COLS
    cstart = [0, a]
    cwidth = [a, F - a]
    outs = []
    for r in res.results:
        arr = np.asarray(r["out"])
        result = np.empty((S, P, F), dtype=np.float16)
        for c, o, g, q, off, t0 in slabs:
            fc = cwidth[c]
            block = arr[off: off + P * g * fc].reshape(P, g, fc)
            result[o: o + g, :, cstart[c]: cstart[c] + fc] = (
                block.transpose(1, 0, 2)
            )
        outs.append(result.reshape(S, VSH, B, H))
    full = np.concatenate(outs, axis=1)              # [S, V, B, H]
    return np.ascontiguousarray(full[..., None].astype(np.float32))
